# revision 1
# baseline (speedup 1.0000x reference)
"""Bass/Tile kernel builder for BSplineField3d (tricubic B-spline interpolation).

Algorithm (per NeuronCore, data-parallel over points):
  Phase 1 (build): from phi [128,128,128,3] build Cy [128x, 125yc, 128z, 3c*4k]
    where Cy[x,yc,z,c,k] = sum_m A[k,m] * phi[x, yc+m, z, c].
    The y-dimension B-spline is thus pre-contracted into per-cell polynomial
    coefficients in v (the fractional y coordinate), making each point's
    required data a CONTIGUOUS 48-float record per x-tap:
        rec(x, yc, z0) = Cy[x, yc, z0:z0+4, :, :]   (z-window x c x k)
    Built with PE matmuls against a constant banded matrix W[y, (k,yc)].
  Phase 2 (points): per chunk of 128x64 points:
    - compute cell indices + fractional coords on DVE
    - indirect DMA gather: 4 records per point (x-taps l=0..3), 192B each
    - combine with weights M[l,n,k] = wu_l * ww_n * v^k on DVE:
        T[c] = sum_{l,n,k} M * R[l, n, c, k]
"""

from contextlib import ExitStack

import sys as _sys
for _p in ("/opt/trn_rl_repo",):
    if _p not in _sys.path:
        _sys.path.append(_p)

import numpy as np

import concourse.bass as bass
import concourse.tile as tile
from concourse import mybir
from concourse._compat import with_exitstack

F32 = mybir.dt.float32
I32 = mybir.dt.int32

NX = 128          # grid points per dim
NCELL = 125       # valid cells per dim (ix in [0,124])
NC_ = 3           # components
ZC = NX * NC_     # 384 floats per (x,y) z-row in phi
KC = 12           # (c=3, k=4) floats per (x,yc,z) in Cy
ROW = NX * KC     # 1536 floats per (x,yc) in Cy
QROW = NX * 48    # 6144 floats per (xq,yc): [z, xs4, c3, k4]
CY_ELEMS = NX * NCELL * ROW  # 24,576,000 (98.3 MB fp32)

COLS = 1984       # points per partition (128*1984 = 253952 >= 250000)
P = 32            # points per partition per chunk
NCHUNK = COLS // P  # 31

# spacing: dx = 2/(nx-3) = 2/125 -> 1/dx = 62.5; u = (x+1)*62.5
INV_D = 62.5
XSTRIDE = NCELL * NX  # 16000: record-index stride for x (in 12-float units)


def bspline_poly_A():
    """A[k][m]: coefficient of v^k in the cubic B-spline weight of tap m."""
    return np.array(
        [
            [1 / 6, 4 / 6, 1 / 6, 0.0],
            [-3 / 6, 0.0, 3 / 6, 0.0],
            [3 / 6, -6 / 6, 3 / 6, 0.0],
            [-1 / 6, 3 / 6, -3 / 6, 1 / 6],
        ],
        dtype=np.float64,
    )


def build_W_const():
    """W[y, k*125+yc] = A[k, y-yc] for 0 <= y-yc <= 3 else 0. Shape [128, 500]."""
    A = bspline_poly_A()
    W = np.zeros((128, 4, 125), np.float32)
    for yc in range(NCELL):
        for m in range(4):
            for k in range(4):
                W[yc + m, k, yc] = A[k, m]
    return W.reshape(128, 500)


def _ap(t, offset, dims):
    """Build a raw AP on the same tensor as AP `t` with explicit [step, num] dims."""
    return bass.AP(tensor=t.tensor, offset=t.offset + offset, ap=[list(d) for d in dims])


@with_exitstack
def bspline_kernel(ctx: ExitStack, tc: tile.TileContext, outs, ins):
    """outs = [T_out [128, COLS, 3] f32]; ins = [xs, ys, zs [128, COLS] f32, phi [128,128,384] f32]."""
    nc = tc.nc
    xs, ys, zs, phi = ins
    t_out = outs[0]

    w_np = build_W_const()
    w_dram = nc.inline_tensor(w_np, name="w_const")

    dram = ctx.enter_context(tc.tile_pool(name="cydram", bufs=1, space="DRAM"))
    cy = dram.tile([NX // 4, NCELL, QROW], F32)

    # ---------------- Phase 1: build Cy ----------------
    with ExitStack() as p1:
        singles = p1.enter_context(tc.tile_pool(name="p1_singles", bufs=1))
        phis = p1.enter_context(tc.tile_pool(name="p1_phi", bufs=3))
        stages = p1.enter_context(tc.tile_pool(name="p1_stage", bufs=3))
        psums = p1.enter_context(tc.psum_pool(name="p1_psum", bufs=8))

        w_sb = singles.tile([128, 500], F32)
        nc.default_dma_engine.dma_start(out=w_sb[:], in_=w_dram.ap())

        for xq in range(NX // 4):
            stage = stages.tile([128, QROW], F32)  # [yc, z*48 + xs*12 + c*4 + k]
            for xsl in range(4):
                phi_x = phis.tile([128, ZC], F32, name=f"phi_{xsl}")
                nc.default_dma_engine.dma_start(out=phi_x[:], in_=phi[4 * xq + xsl])
                for k in range(4):
                    ps = psums.tile([NCELL, ZC], F32)
                    nc.tensor.matmul(
                        ps[:],
                        w_sb[:, k * NCELL:(k + 1) * NCELL],
                        phi_x[:],
                        start=True,
                        stop=True,
                    )
                    # psum [yc, (z,c)] -> stage[yc, z*48 + xs*12 + c*4 + k]
                    eng = nc.scalar if (k % 2 == 0) else nc.vector
                    src = _ap(ps[:], 0, [[ZC, NCELL], [3, NX], [1, NC_]])
                    dst = _ap(stage[:], xsl * KC + k,
                              [[QROW, NCELL], [48, NX], [4, NC_]])
                    if eng is nc.scalar:
                        eng.copy(out=dst, in_=src)
                    else:
                        eng.tensor_copy(out=dst, in_=src)
            nc.default_dma_engine.dma_start(
                out=cy[xq, :, :], in_=stage[:NCELL, :]
            )

    # ---------------- Phase 2: points ----------------
    with ExitStack() as p2:
        sing = p2.enter_context(tc.tile_pool(name="p2_singles", bufs=1))
        coords = p2.enter_context(tc.tile_pool(name="p2_coords", bufs=3))
        small = p2.enter_context(tc.tile_pool(name="p2_small", bufs=2))
        idxp = p2.enter_context(tc.tile_pool(name="p2_idx", bufs=3))
        recs = p2.enter_context(tc.tile_pool(name="p2_rec", bufs=2))
        prods = p2.enter_context(tc.tile_pool(name="p2_prod", bufs=2))
        touts = p2.enter_context(tc.tile_pool(name="p2_tout", bufs=2))

        # j-ramp constant: [128, 8] = 0..7 (x-slot index within gathered pair)
        jr8 = sing.tile([128, 8], F32)
        nc.gpsimd.iota(jr8[:], [[1, 8]], channel_multiplier=0,
                       allow_small_or_imprecise_dtypes=True)

        cy_flat = _ap(cy[:], 0, [[48, CY_ELEMS // 48], [1, 48]])

        for ch in range(NCHUNK):
            x_t = coords.tile([128, P], F32)
            y_t = coords.tile([128, P], F32)
            z_t = coords.tile([128, P], F32)
            nc.default_dma_engine.dma_start(out=x_t[:], in_=xs[:, ch * P:(ch + 1) * P])
            nc.default_dma_engine.dma_start(out=y_t[:], in_=ys[:, ch * P:(ch + 1) * P])
            nc.default_dma_engine.dma_start(out=z_t[:], in_=zs[:, ch * P:(ch + 1) * P])

            # --- cell indices + fractions (DVE) ---
            def exact_floor(src, out, sfx):
                # out = floor(src) for src >= 0, robust to cast rounding mode
                ci = small.tile([128, P], I32, name=f"ci_{sfx}")
                cf = small.tile([128, P], F32, name=f"cf_{sfx}")
                nc.vector.tensor_copy(out=ci[:], in_=src[:])
                nc.vector.tensor_copy(out=cf[:], in_=ci[:])
                nc.vector.tensor_tensor(out[:], cf[:], src[:], mybir.AluOpType.is_gt)
                nc.vector.tensor_sub(out[:], cf[:], out[:])

            def split_coord(src, sfx):
                u = small.tile([128, P], F32, name=f"u_{sfx}")
                fr = small.tile([128, P], F32, name=f"fr_{sfx}")
                ii = small.tile([128, P], F32, name=f"ii_{sfx}")
                nc.vector.tensor_scalar(u[:], src[:], 1.0, INV_D,
                                        mybir.AluOpType.add, mybir.AluOpType.mult)
                exact_floor(u, ii, sfx)
                nc.vector.tensor_sub(fr[:], u[:], ii[:])
                nc.vector.tensor_scalar(ii[:], ii[:], float(NCELL - 1), 0.0,
                                        mybir.AluOpType.min, mybir.AluOpType.max)
                return ii, fr

            ix_t, fu = split_coord(x_t, "x")
            iy_t, fv = split_coord(y_t, "y")
            iz_t, fw = split_coord(z_t, "z")

            # --- x-quad decomposition: xq = ix//4, s0 = ix%4, xq1 = min(xq+1,31)
            tq = small.tile([128, P], F32)
            xq_t = small.tile([128, P], F32)
            xq1_t = small.tile([128, P], F32)
            s0_t = small.tile([128, P], F32)
            nc.vector.tensor_scalar(tq[:], ix_t[:], 0.25, None, mybir.AluOpType.mult)
            exact_floor(tq, xq_t, "q")
            nc.vector.scalar_tensor_tensor(
                s0_t[:], xq_t[:], -4.0, ix_t[:],
                mybir.AluOpType.mult, mybir.AluOpType.add)
            nc.vector.tensor_scalar(xq1_t[:], xq_t[:], 1.0, 31.0,
                                    mybir.AluOpType.add, mybir.AluOpType.min)

            # --- record indices (48-float units): xqg*16000 + iy*128 + iz
            byz = small.tile([128, P], F32)
            nc.vector.scalar_tensor_tensor(
                byz[:], iy_t[:], float(NX), iz_t[:],
                mybir.AluOpType.mult, mybir.AluOpType.add)
            idx_f = idxp.tile([128, P, 2], F32)
            nc.vector.scalar_tensor_tensor(
                _ap(idx_f[:], 0, [[P * 2, 128], [2, P]]),
                xq_t[:], float(XSTRIDE), byz[:],
                mybir.AluOpType.mult, mybir.AluOpType.add)
            nc.vector.scalar_tensor_tensor(
                _ap(idx_f[:], 1, [[P * 2, 128], [2, P]]),
                xq1_t[:], float(XSTRIDE), byz[:],
                mybir.AluOpType.mult, mybir.AluOpType.add)
            idx_i = idxp.tile([128, P * 2], I32)
            nc.vector.tensor_copy(out=idx_i[:], in_=idx_f[:].rearrange("p a b -> p (a b)"))

            # --- gather: 2 records x 192 floats per point (vector-mode: one
            # descriptor per partition per instruction) ---
            rec = recs.tile([128, P * 2 * 192], F32)
            for t in range(P * 2):
                nc.gpsimd.indirect_dma_start(
                    out=_ap(rec[:], t * 192, [[P * 2 * 192, 128], [1, 192]]),
                    out_offset=None,
                    in_=cy_flat,
                    in_offset=bass.IndirectOffsetOnAxis(
                        ap=_ap(idx_i[:], t, [[P * 2, 128], [1, 1]]), axis=0),
                )

            # --- weights ---
            def tap_weights(fr, sfx):
                wt = small.tile([128, P, 4], F32, name=f"wt_{sfx}")
                t = small.tile([128, P], F32, name=f"t_{sfx}")
                t2 = small.tile([128, P], F32, name=f"t2_{sfx}")
                r2 = small.tile([128, P], F32, name=f"r2_{sfx}")
                r3 = small.tile([128, P], F32, name=f"r3_{sfx}")
                w0 = wt[:, :, 0]
                w1 = wt[:, :, 1]
                w2 = wt[:, :, 2]
                w3 = wt[:, :, 3]
                nc.vector.tensor_scalar(t[:], fr[:], -1.0, 1.0,
                                        mybir.AluOpType.mult, mybir.AluOpType.add)
                nc.vector.tensor_mul(t2[:], t[:], t[:])
                nc.vector.scalar_tensor_tensor(w0, t2[:], 1 / 6, t[:],
                                               mybir.AluOpType.mult, mybir.AluOpType.mult)
                nc.vector.tensor_mul(r2[:], fr[:], fr[:])
                nc.vector.tensor_mul(r3[:], r2[:], fr[:])
                nc.vector.tensor_scalar(w3, r3[:], 1 / 6, None, mybir.AluOpType.mult)
                nc.vector.scalar_tensor_tensor(w1, r3[:], 0.5, r2[:],
                                               mybir.AluOpType.mult, mybir.AluOpType.subtract)
                nc.vector.tensor_scalar(w1, w1, 2 / 3, None, mybir.AluOpType.add)
                nc.vector.tensor_add(w2, w0, w1)
                nc.vector.tensor_add(w2, w2, w3)
                nc.vector.tensor_scalar(w2, w2, -1.0, 1.0,
                                        mybir.AluOpType.mult, mybir.AluOpType.add)
                return wt

            wu = tap_weights(fu, "u")
            ww = tap_weights(fw, "w")

            vp = small.tile([128, P, 4], F32)
            nc.vector.memset(vp[:, :, 0], 1.0)
            nc.vector.tensor_copy(out=vp[:, :, 1], in_=fv[:])
            nc.vector.tensor_mul(vp[:, :, 2], fv[:], fv[:])
            nc.vector.tensor_mul(vp[:, :, 3], vp[:, :, 2], fv[:])

            # --- W8[pt, j] = wu[j - s0] for j-s0 in [0,4) else 0 ---
            d8 = small.tile([128, P, 8], F32)
            e8 = small.tile([128, P, 8], F32)
            w8 = small.tile([128, P, 8], F32)
            nc.vector.tensor_tensor(
                _ap(d8[:], 0, [[P * 8, 128], [8, P], [1, 8]]),
                _ap(jr8[:], 0, [[8, 128], [0, P], [1, 8]]),
                _ap(s0_t[:], 0, [[P, 128], [1, P], [0, 8]]),
                mybir.AluOpType.subtract)
            for l in range(4):
                tgt = w8 if l == 0 else e8
                nc.vector.tensor_scalar(e8[:], d8[:], float(l), None,
                                        mybir.AluOpType.is_equal)
                nc.vector.tensor_tensor(
                    _ap(tgt[:], 0, [[P * 8, 128], [8, P], [1, 8]]),
                    _ap(e8[:], 0, [[P * 8, 128], [8, P], [1, 8]]),
                    _ap(wu[:], l, [[P * 4, 128], [4, P], [0, 8]]),
                    mybir.AluOpType.mult)
                if l > 0:
                    nc.vector.tensor_add(w8[:], w8[:], e8[:])

            # --- contraction: T[c] = sum_{g,z,x,k} w8[gx]*ww[z]*v^k * R ---
            # rec per point: [g2][z4][x4][c3][k4] (gzxc=96, k innermost)
            # 1) contract k with vp (in-place into rec), reduce -> s1 [gzxc]
            nc.vector.tensor_tensor(
                _ap(rec[:], 0, [[P * 384, 128], [384, P], [4, 96], [1, 4]]),
                _ap(rec[:], 0, [[P * 384, 128], [384, P], [4, 96], [1, 4]]),
                _ap(vp[:], 0, [[P * 4, 128], [4, P], [0, 96], [1, 4]]),
                mybir.AluOpType.mult)
            s1 = prods.tile([128, P * 96], F32)
            nc.vector.tensor_reduce(
                out=s1[:],
                in_=_ap(rec[:], 0, [[P * 384, 128], [4, P * 96], [1, 4]]),
                axis=mybir.AxisListType.X,
                op=mybir.AluOpType.add)
            # 2) contract z with ww; write transposed so z is innermost
            t1 = prods.tile([128, P * 96], F32)
            for g in range(2):
                nc.vector.tensor_tensor(
                    _ap(t1[:], g * 48, [[P * 96, 128], [96, P], [1, 4], [4, 12]]),
                    _ap(s1[:], g * 48, [[P * 96, 128], [96, P], [12, 4], [1, 12]]),
                    _ap(ww[:], 0, [[P * 4, 128], [4, P], [1, 4], [0, 12]]),
                    mybir.AluOpType.mult)
            s2 = touts.tile([128, P * 24], F32)
            nc.vector.tensor_reduce(
                out=s2[:],
                in_=_ap(t1[:], 0, [[P * 96, 128], [4, P * 24], [1, 4]]),
                axis=mybir.AxisListType.X,
                op=mybir.AluOpType.add)
            # 3) contract (g,x) with w8; write transposed so gx is innermost
            t2 = touts.tile([128, P * 24], F32)
            nc.vector.tensor_tensor(
                _ap(t2[:], 0, [[P * 24, 128], [24, P], [1, 8], [8, 3]]),
                _ap(s2[:], 0, [[P * 24, 128], [24, P], [3, 8], [1, 3]]),
                _ap(w8[:], 0, [[P * 8, 128], [8, P], [1, 8], [0, 3]]),
                mybir.AluOpType.mult)
            t_c = touts.tile([128, P * 3], F32)
            nc.vector.tensor_reduce(
                out=t_c[:],
                in_=_ap(t2[:], 0, [[P * 24, 128], [8, P * 3], [1, 8]]),
                axis=mybir.AxisListType.X,
                op=mybir.AluOpType.add)

            nc.default_dma_engine.dma_start(
                out=t_out[:, ch * P:(ch + 1) * P, :],
                in_=t_c[:].rearrange("p (a b) -> p a b", b=3))


# ======================================================================
# Self-contained entry point: kernel(**inputs) -> np.ndarray
# ======================================================================

N_POINTS = 2_000_000
N_CORES = 8
PTS_PER_CORE = N_POINTS // N_CORES      # 250000
PAD_PER_CORE = 128 * COLS               # 253952

_CACHE = {}


def _build_nc():
    import concourse.bacc as bacc

    nc = bacc.Bacc(
        "TRN2",
        target_bir_lowering=False,
        debug=False,
        num_devices=N_CORES,
    )
    xs = nc.dram_tensor("xs", [128, COLS], F32, kind="ExternalInput").ap()
    ys = nc.dram_tensor("ys", [128, COLS], F32, kind="ExternalInput").ap()
    zs = nc.dram_tensor("zs", [128, COLS], F32, kind="ExternalInput").ap()
    phi = nc.dram_tensor("phi", [128, 128, ZC], F32, kind="ExternalInput").ap()
    t_out = nc.dram_tensor("t_out", [128, COLS, NC_], F32, kind="ExternalOutput").ap()

    with tile.TileContext(nc) as tc:
        bspline_kernel(tc, [t_out], [xs, ys, zs, phi])
    nc.compile()
    return nc


def get_nc():
    if "nc" not in _CACHE:
        _CACHE["nc"] = _build_nc()
    return _CACHE["nc"]


def _shard(arr):
    """[N_POINTS] -> list of 8 [128, COLS] arrays (padded with zeros)."""
    out = []
    for c in range(N_CORES):
        s = arr[c * PTS_PER_CORE:(c + 1) * PTS_PER_CORE]
        p = np.zeros(PAD_PER_CORE, dtype=np.float32)
        p[:PTS_PER_CORE] = s
        out.append(p.reshape(128, COLS))
    return out


def run_on_cores(x, y, z, phi_x, trace=False, **kw):
    from concourse.bass_utils import run_bass_kernel_spmd

    nc = get_nc()
    xsh, ysh, zsh = _shard(x), _shard(y), _shard(z)
    phi_r = np.ascontiguousarray(phi_x.reshape(128, 128, ZC))
    in_maps = [
        {"xs": xsh[c], "ys": ysh[c], "zs": zsh[c], "phi": phi_r}
        for c in range(N_CORES)
    ]
    res = run_bass_kernel_spmd(
        nc, in_maps, core_ids=list(range(N_CORES)), trace=trace, **kw
    )
    outs = []
    for c in range(N_CORES):
        t = res.results[c]["t_out"].reshape(PAD_PER_CORE, NC_)
        outs.append(t[:PTS_PER_CORE])
    full = np.concatenate(outs, axis=0).astype(np.float32)
    return full, res


def kernel(x, y, z, phi_x):
    full, _ = run_on_cores(
        np.asarray(x, dtype=np.float32),
        np.asarray(y, dtype=np.float32),
        np.asarray(z, dtype=np.float32),
        np.asarray(phi_x, dtype=np.float32),
    )
    return full



# revision 6
# speedup vs baseline: 1.7503x; 1.7503x over previous
"""Bass/Tile kernel builder for BSplineField3d (tricubic B-spline interpolation).

Algorithm (per NeuronCore, data-parallel over points):
  Phase 1 (build): from phi [128,128,128,3] build Cy4 in fp16:
      Cy4[x0, yc, z, xs, c, k] = sum_m A[k,m] * phi[x0+xs, yc+m, z, c]
    (x0 in [0,124], xs in [0,4)).  The y-dim B-spline is pre-contracted into
    per-cell polynomial coefficients in v; the 4 x-taps of a point are
    DUPLICATED into every record so that one point needs exactly ONE
    contiguous gather: records of 48 fp16 = [xs4][c3][k4] are contiguous
    along z, so the z-window (4 records = 192 fp16 = 384 B) starting at
    (x0=ix, yc=iy, z=iz) holds everything point-specific.
    Built with fp16 PE matmuls against a banded matrix W[y,(k,yc)], with a
    sliding window of stage tiles (each x-slab feeds 4 stages).
  Phase 2 (points): per chunk of 128x128 points:
    - cell indices + fractional coords on DVE
    - P indirect-DMA gathers (one index per partition per instruction,
      the only vector-mode the HW ucode supports), 384 B per descriptor
    - contraction on DVE in fp16 (packed APs -> 2x perf mode):
        poly-eval in v over k (mult + tree-add), weighted x taps,
        weighted z taps (tree-adds, partially in-place)
"""

from contextlib import ExitStack

import sys as _sys
for _p in ("/opt/trn_rl_repo",):
    if _p not in _sys.path:
        _sys.path.append(_p)

import numpy as np

import concourse.bass as bass
import concourse.tile as tile
from concourse import mybir
from concourse._compat import with_exitstack

F32 = mybir.dt.float32
F16 = mybir.dt.float16
I32 = mybir.dt.int32

NX = 128          # grid points per dim
NCELL = 125       # valid cells per dim (ix in [0,124])
NC_ = 3           # components
ZC = NX * NC_     # 384 floats per (x,y) z-row in phi
REC = 48          # [xs4][c3][k4] fp16 per (x0,yc,z) record in Cy4
ROWE = NX * REC   # 6144 fp16 per (x0,yc)
NRECTOT = NCELL * NCELL * NX   # 2,000,000 records
XSTRIDE = NCELL * NX           # 16000: record-index stride for x0

COLS = 2048       # points per partition (128*2048 = 262144 >= 250000)
P = 128           # points per partition per chunk
NCHUNK = COLS // P  # 16

# spacing: dx = 2/(nx-3) = 2/125 -> 1/dx = 62.5; u = (x+1)*62.5
INV_D = 62.5


def bspline_poly_A():
    """A[k][m]: coefficient of v^k in the cubic B-spline weight of tap m."""
    return np.array(
        [
            [1 / 6, 4 / 6, 1 / 6, 0.0],
            [-3 / 6, 0.0, 3 / 6, 0.0],
            [3 / 6, -6 / 6, 3 / 6, 0.0],
            [-1 / 6, 3 / 6, -3 / 6, 1 / 6],
        ],
        dtype=np.float64,
    )


def build_W_const():
    """W[y, k*125+yc] = A[k, y-yc] for 0 <= y-yc <= 3 else 0. Shape [128, 500]."""
    A = bspline_poly_A()
    W = np.zeros((128, 4, 125), np.float32)
    for yc in range(NCELL):
        for m in range(4):
            for k in range(4):
                W[yc + m, k, yc] = A[k, m]
    return W.reshape(128, 500).astype(np.float16)


def _ap(t, offset, dims):
    """Build a raw AP on the same tensor as AP `t` with explicit [step, num] dims."""
    return bass.AP(tensor=t.tensor, offset=t.offset + offset, ap=[list(d) for d in dims])


@with_exitstack
def bspline_kernel(ctx: ExitStack, tc: tile.TileContext, outs, ins):
    """outs = [T_out [128, COLS, 3] f32]; ins = [xs, ys, zs [128, COLS] f32, phi [128,128,384] f32]."""
    nc = tc.nc
    xs, ys, zs, phi = ins
    t_out = outs[0]

    w_np = build_W_const()
    w_dram = nc.inline_tensor(w_np, name="w_const")

    dram = ctx.enter_context(tc.tile_pool(name="cydram", bufs=1, space="DRAM"))
    cy = dram.tile([NRECTOT, REC], F16)

    add = mybir.AluOpType.add
    sub = mybir.AluOpType.subtract
    mult = mybir.AluOpType.mult
    amin = mybir.AluOpType.min

    # ---------------- Phase 1: build Cy4 ----------------
    with ExitStack() as p1:
        singles = p1.enter_context(tc.tile_pool(name="p1_singles", bufs=1))
        phis = p1.enter_context(tc.tile_pool(name="p1_phi", bufs=3))
        stages = p1.enter_context(tc.tile_pool(name="p1_stage", bufs=6))
        psums = p1.enter_context(tc.psum_pool(name="p1_psum", bufs=2))

        w_sb = singles.tile([128, 500], F16)
        nc.sync.dma_start(out=w_sb[:], in_=w_dram.ap())

        stage_by_x0 = {}
        for x in range(NX):
            phi_x = phis.tile([128, ZC], F16, name="phi_in")
            # cast fp32 -> fp16 during DMA (SWDGE)
            nc.gpsimd.dma_start(out=phi_x[:], in_=phi[x])
            ps = psums.tile([NCELL, 2048], F32)
            for k in range(4):
                nc.tensor.matmul(
                    ps[:, k * 512:k * 512 + ZC],
                    w_sb[:, k * NCELL:(k + 1) * NCELL],
                    phi_x[:],
                    start=True,
                    stop=True,
                )
            for xsl in range(4):
                x0 = x - xsl
                if x0 < 0 or x0 > NCELL - 1:
                    continue
                if xsl == 0:
                    stage_by_x0[x0] = stages.tile([128, ROWE], F16, name="stage")
                stage = stage_by_x0[x0]
                # psum [yc | k4, (z,c)] -> stage[yc | z, xsl, c, k]
                if (x + xsl) % 2 == 0:
                    src = _ap(ps[:], 0, [[2048, NCELL], [512, 4], [3, NX], [1, NC_]])
                    dst = _ap(stage[:], xsl * 12,
                              [[ROWE, NCELL], [1, 4], [REC, NX], [4, NC_]])
                    nc.vector.tensor_copy(out=dst, in_=src)
                else:
                    for k in range(4):
                        src = _ap(ps[:], k * 512, [[2048, NCELL], [3, NX], [1, NC_]])
                        dst = _ap(stage[:], xsl * 12 + k,
                                  [[ROWE, NCELL], [REC, NX], [4, NC_]])
                        nc.scalar.copy(out=dst, in_=src)
                if xsl == 3 or (x == NX - 1 and xsl == 0):
                    # stage complete (x0+3 == x, or tail stages at the last x)
                    pass
            # ship completed stages: stage x0 is complete once x == x0+3
            ship = []
            if x >= 3:
                ship.append(x - 3)
            if x == NX - 1:
                ship.extend([NCELL - 3, NCELL - 2, NCELL - 1])
            for x0 in ship:
                if x0 not in stage_by_x0:
                    continue
                st = stage_by_x0.pop(x0)
                eng = nc.sync if (x0 % 2 == 0) else nc.gpsimd
                eng.dma_start(
                    out=_ap(cy[:], x0 * XSTRIDE * REC,
                            [[ROWE, NCELL], [1, ROWE]]),
                    in_=_ap(st[:], 0, [[ROWE, NCELL], [1, ROWE]]),
                )

    # ---------------- Phase 2: points ----------------
    with ExitStack() as p2:
        coords = p2.enter_context(tc.tile_pool(name="p2_coords", bufs=2))
        small = p2.enter_context(tc.tile_pool(name="p2_small", bufs=2))
        idxp = p2.enter_context(tc.tile_pool(name="p2_idx", bufs=2))
        recs = p2.enter_context(tc.tile_pool(name="p2_rec", bufs=2))
        prods = p2.enter_context(tc.tile_pool(name="p2_prod", bufs=2))
        touts = p2.enter_context(tc.tile_pool(name="p2_tout", bufs=2))

        cy_flat = _ap(cy[:], 0, [[REC, NRECTOT], [1, REC]])

        for ch in range(NCHUNK):
            # coords layout: [x | z | y] so (u,w) are adjacent for weights
            c3 = coords.tile([128, 3 * P], F32)
            nc.sync.dma_start(out=c3[:, 0:P], in_=xs[:, ch * P:(ch + 1) * P])
            nc.sync.dma_start(out=c3[:, P:2 * P], in_=zs[:, ch * P:(ch + 1) * P])
            nc.sync.dma_start(out=c3[:, 2 * P:3 * P], in_=ys[:, ch * P:(ch + 1) * P])

            # u = (coord+1)*62.5 ; fl = floor(u) (u >= 0) ; fr = u - fl
            nc.vector.tensor_scalar(c3[:], c3[:], 1.0, INV_D, add, mult)
            ci3 = small.tile([128, 3 * P], I32)
            nc.vector.tensor_copy(out=ci3[:], in_=c3[:])
            cf3 = small.tile([128, 3 * P], F32)
            nc.vector.tensor_copy(out=cf3[:], in_=ci3[:])
            fl3 = small.tile([128, 3 * P], F32)
            nc.vector.tensor_tensor(fl3[:], cf3[:], c3[:], mybir.AluOpType.is_gt)
            nc.vector.tensor_tensor(fl3[:], cf3[:], fl3[:], sub)
            fr3 = small.tile([128, 3 * P], F32)
            nc.vector.tensor_tensor(fr3[:], c3[:], fl3[:], sub)
            nc.vector.tensor_scalar(fl3[:], fl3[:], float(NCELL - 1), None, amin)
            frh = small.tile([128, 3 * P], F16)
            nc.vector.tensor_copy(out=frh[:], in_=fr3[:])

            # record index: ix*16000 + iy*128 + iz
            idxf = small.tile([128, P], F32)
            nc.vector.scalar_tensor_tensor(
                idxf[:], fl3[:, 2 * P:3 * P], float(NX), fl3[:, P:2 * P], mult, add)
            nc.vector.scalar_tensor_tensor(
                idxf[:], fl3[:, 0:P], float(XSTRIDE), idxf[:], mult, add)
            idxi = idxp.tile([128, P], I32)
            nc.vector.tensor_copy(out=idxi[:], in_=idxf[:])

            # ---- tap weights for u (x) and w (z): wt [128 | g2, P, k4] fp16 ----
            wt = small.tile([128, 2 * P * 4], F16)
            uw = _ap(frh[:], 0, [[3 * P, 128], [1, 2 * P]])

            def wslice(k):
                return _ap(wt[:], k, [[8 * P, 128], [4 * P, 2], [4, P]])

            tg = small.tile([128, 2 * P], F16)
            t2g = small.tile([128, 2 * P], F16)
            r2 = small.tile([128, 2 * P], F16)
            r3 = small.tile([128, 2 * P], F16)
            tmp = small.tile([128, 2 * P], F16)

            def v2(t):  # view [128, 2P] as (2, P)
                return _ap(t[:], 0, [[2 * P, 128], [P, 2], [1, P]])

            nc.vector.tensor_scalar(tg[:], uw, -1.0, 1.0, mult, add)
            nc.vector.tensor_tensor(t2g[:], tg[:], tg[:], mult)
            nc.vector.scalar_tensor_tensor(wslice(0), v2(t2g), 1 / 6, v2(tg), mult, mult)
            nc.vector.tensor_tensor(r2[:], uw, uw, mult)
            nc.vector.tensor_tensor(r3[:], r2[:], uw, mult)
            nc.vector.tensor_scalar(wslice(3), v2(r3), 1 / 6, None, mult)
            nc.vector.scalar_tensor_tensor(tmp[:], r3[:], 0.5, r2[:], mult, sub)
            nc.vector.tensor_scalar(wslice(1), v2(tmp), 2 / 3, None, add)
            nc.vector.tensor_tensor(v2(tmp), wslice(0), wslice(1), add)
            nc.vector.tensor_tensor(v2(tmp), v2(tmp), wslice(3), add)
            nc.vector.tensor_scalar(wslice(2), v2(tmp), -1.0, 1.0, mult, add)

            # ---- v powers: vp4 [128, P, 4] = [1, v, v^2, v^3] fp16 ----
            vp4 = small.tile([128, P * 4], F16)
            frv = _ap(frh[:], 2 * P, [[3 * P, 128], [1, P]])

            def vslot(k):
                return _ap(vp4[:], k, [[4 * P, 128], [4, P]])

            nc.vector.memset(vslot(0), 1.0)
            nc.vector.tensor_copy(out=vslot(1), in_=frv)
            nc.vector.tensor_tensor(vslot(2), frv, frv, mult)
            nc.vector.tensor_tensor(vslot(3), vslot(2), frv, mult)

            # ---- x weights expanded over c: wuc [128, P, x4, c3] fp16 ----
            wuc = small.tile([128, P * 12], F16)
            nc.vector.tensor_copy(
                out=_ap(wuc[:], 0, [[12 * P, 128], [12, P], [3, 4], [1, 3]]),
                in_=_ap(wt[:], 0, [[8 * P, 128], [4, P], [1, 4], [0, 3]]))

            # ---- gather: one record (z-window, 192 fp16) per point ----
            rec = recs.tile([128, P * 192], F16)
            for t in range(P):
                nc.gpsimd.indirect_dma_start(
                    out=_ap(rec[:], t * 192, [[192 * P, 128], [1, 192]]),
                    out_offset=None,
                    in_=cy_flat,
                    in_offset=bass.IndirectOffsetOnAxis(
                        ap=_ap(idxi[:], t, [[P, 128], [1, 1]]), axis=0),
                )

            # ---- contraction ----
            # per point rec = [z4][x4][c3][k4]
            # k poly-eval: rec[pt, zxc48, k4] *= vp4[pt, k4]; tree-add over k
            nc.vector.tensor_tensor(
                _ap(rec[:], 0, [[192 * P, 128], [192, P], [4, 48], [1, 4]]),
                _ap(rec[:], 0, [[192 * P, 128], [192, P], [4, 48], [1, 4]]),
                _ap(vp4[:], 0, [[4 * P, 128], [4, P], [0, 48], [1, 4]]),
                mult)
            nc.vector.tensor_tensor(
                _ap(rec[:], 0, [[192 * P, 128], [192, P], [4, 48], [1, 2]]),
                _ap(rec[:], 0, [[192 * P, 128], [192, P], [4, 48], [1, 2]]),
                _ap(rec[:], 2, [[192 * P, 128], [192, P], [4, 48], [1, 2]]),
                add)
            s1 = prods.tile([128, P * 48], F16)
            nc.vector.tensor_tensor(
                _ap(s1[:], 0, [[48 * P, 128], [48, P], [1, 48]]),
                _ap(rec[:], 0, [[192 * P, 128], [192, P], [4, 48]]),
                _ap(rec[:], 1, [[192 * P, 128], [192, P], [4, 48]]),
                add)
            # x contraction: s1[pt, z4, (x4 c3)12] *= wuc; tree-add over x
            nc.vector.tensor_tensor(
                _ap(s1[:], 0, [[48 * P, 128], [48, P], [12, 4], [1, 12]]),
                _ap(s1[:], 0, [[48 * P, 128], [48, P], [12, 4], [1, 12]]),
                _ap(wuc[:], 0, [[12 * P, 128], [12, P], [0, 4], [1, 12]]),
                mult)
            nc.vector.tensor_tensor(
                _ap(s1[:], 0, [[48 * P, 128], [48, P], [12, 4], [1, 6]]),
                _ap(s1[:], 0, [[48 * P, 128], [48, P], [12, 4], [1, 6]]),
                _ap(s1[:], 6, [[48 * P, 128], [48, P], [12, 4], [1, 6]]),
                add)
            s2 = prods.tile([128, P * 12], F16)
            nc.vector.tensor_tensor(
                _ap(s2[:], 0, [[12 * P, 128], [12, P], [3, 4], [1, 3]]),
                _ap(s1[:], 0, [[48 * P, 128], [48, P], [12, 4], [1, 3]]),
                _ap(s1[:], 3, [[48 * P, 128], [48, P], [12, 4], [1, 3]]),
                add)
            # z contraction: s2[pt, z4, c3] *= ww (bcast over c); tree-add over z
            nc.vector.tensor_tensor(
                _ap(s2[:], 0, [[12 * P, 128], [12, P], [3, 4], [1, 3]]),
                _ap(s2[:], 0, [[12 * P, 128], [12, P], [3, 4], [1, 3]]),
                _ap(wt[:], 4 * P, [[8 * P, 128], [4, P], [1, 4], [0, 3]]),
                mult)
            nc.vector.tensor_tensor(
                _ap(s2[:], 0, [[12 * P, 128], [12, P], [1, 6]]),
                _ap(s2[:], 0, [[12 * P, 128], [12, P], [1, 6]]),
                _ap(s2[:], 6, [[12 * P, 128], [12, P], [1, 6]]),
                add)
            t_c = touts.tile([128, P * 3], F32)
            nc.vector.tensor_tensor(
                _ap(t_c[:], 0, [[3 * P, 128], [3, P], [1, 3]]),
                _ap(s2[:], 0, [[12 * P, 128], [12, P], [1, 3]]),
                _ap(s2[:], 3, [[12 * P, 128], [12, P], [1, 3]]),
                add)

            nc.sync.dma_start(
                out=t_out[:, ch * P:(ch + 1) * P, :],
                in_=t_c[:].rearrange("p (a b) -> p a b", b=3))


# ======================================================================
# Self-contained entry point: kernel(**inputs) -> np.ndarray
# ======================================================================

N_POINTS = 2_000_000
N_CORES = 8
PTS_PER_CORE = N_POINTS // N_CORES      # 250000
PAD_PER_CORE = 128 * COLS               # 262144

_CACHE = {}


def _build_nc():
    import concourse.bacc as bacc

    nc = bacc.Bacc(
        "TRN2",
        target_bir_lowering=False,
        debug=False,
        num_devices=N_CORES,
    )
    xs = nc.dram_tensor("xs", [128, COLS], F32, kind="ExternalInput").ap()
    ys = nc.dram_tensor("ys", [128, COLS], F32, kind="ExternalInput").ap()
    zs = nc.dram_tensor("zs", [128, COLS], F32, kind="ExternalInput").ap()
    phi = nc.dram_tensor("phi", [128, 128, ZC], F32, kind="ExternalInput").ap()
    t_out = nc.dram_tensor("t_out", [128, COLS, NC_], F32, kind="ExternalOutput").ap()

    with tile.TileContext(nc) as tc:
        bspline_kernel(tc, [t_out], [xs, ys, zs, phi])
    nc.compile()
    return nc


def get_nc():
    if "nc" not in _CACHE:
        _CACHE["nc"] = _build_nc()
    return _CACHE["nc"]


def _shard(arr):
    """[N_POINTS] -> list of 8 [128, COLS] arrays (padded with zeros)."""
    out = []
    for c in range(N_CORES):
        s = arr[c * PTS_PER_CORE:(c + 1) * PTS_PER_CORE]
        p = np.zeros(PAD_PER_CORE, dtype=np.float32)
        p[:PTS_PER_CORE] = s
        out.append(p.reshape(128, COLS))
    return out


def run_on_cores(x, y, z, phi_x, trace=False, **kw):
    from concourse.bass_utils import run_bass_kernel_spmd

    nc = get_nc()
    xsh, ysh, zsh = _shard(x), _shard(y), _shard(z)
    phi_r = np.ascontiguousarray(phi_x.reshape(128, 128, ZC))
    in_maps = [
        {"xs": xsh[c], "ys": ysh[c], "zs": zsh[c], "phi": phi_r}
        for c in range(N_CORES)
    ]
    res = run_bass_kernel_spmd(
        nc, in_maps, core_ids=list(range(N_CORES)), trace=trace, **kw
    )
    outs = []
    for c in range(N_CORES):
        t = res.results[c]["t_out"].reshape(PAD_PER_CORE, NC_)
        outs.append(t[:PTS_PER_CORE])
    full = np.concatenate(outs, axis=0).astype(np.float32)
    return full, res


def kernel(x, y, z, phi_x):
    full, _ = run_on_cores(
        np.asarray(x, dtype=np.float32),
        np.asarray(y, dtype=np.float32),
        np.asarray(z, dtype=np.float32),
        np.asarray(phi_x, dtype=np.float32),
    )
    return full


# revision 10
# speedup vs baseline: 2.0496x; 1.1710x over previous
"""Bass/Tile kernel builder for BSplineField3d (tricubic B-spline interpolation).

Algorithm (per NeuronCore, data-parallel over points):
  Phase 1 (build): from phi [128,128,128,3] build Cy4 in fp16:
      Cy4[x0, yc, z, xs, c, k] = sum_m A[k,m] * phi[x0+xs, yc+m, z, c]
    (x0 in [0,124], xs in [0,4)).  The y-dim B-spline is pre-contracted into
    per-cell polynomial coefficients in v; the 4 x-taps of a point are
    DUPLICATED into every record so that one point needs exactly ONE
    contiguous gather: records of 48 fp16 = [xs4][c3][k4] are contiguous
    along z, so the z-window (4 records = 192 fp16 = 384 B) starting at
    (x0=ix, yc=iy, z=iz) holds everything point-specific.
    Built with fp16 PE matmuls against a banded matrix W[y,(k,yc)], with a
    sliding window of stage tiles (each x-slab feeds 4 stages).
  Phase 2 (points): per chunk of 128x128 points:
    - cell indices + fractional coords on DVE
    - P indirect-DMA gathers (one index per partition per instruction,
      the only vector-mode the HW ucode supports), 384 B per descriptor
    - contraction on DVE in fp16 (packed APs -> 2x perf mode):
        poly-eval in v over k (mult + tree-add), weighted x taps,
        weighted z taps (tree-adds, partially in-place)
"""

from contextlib import ExitStack

import sys as _sys
for _p in ("/opt/trn_rl_repo",):
    if _p not in _sys.path:
        _sys.path.append(_p)

import numpy as np

import concourse.bass as bass
import concourse.tile as tile
from concourse import mybir
from concourse._compat import with_exitstack

F32 = mybir.dt.float32
F16 = mybir.dt.float16
I32 = mybir.dt.int32

NX = 128          # grid points per dim
NCELL = 125       # valid cells per dim (ix in [0,124])
NC_ = 3           # components
ZC = NX * NC_     # 384 floats per (x,y) z-row in phi
REC = 48          # [xs4][c3][k4] fp16 per (x0,yc,z) record in Cy4
ROWE = NX * REC   # 6144 fp16 per (x0,yc)
NRECTOT = NCELL * NCELL * NX   # 2,000,000 records
XSTRIDE = NCELL * NX           # 16000: record-index stride for x0

COLS = 1984       # points per partition (128*1984 = 253952 >= 250000)
P = 124           # points per partition per chunk
NCHUNK = COLS // P  # 16

# spacing: dx = 2/(nx-3) = 2/125 -> 1/dx = 62.5; u = (x+1)*62.5
INV_D = 62.5


def bspline_poly_A():
    """A[k][m]: coefficient of v^k in the cubic B-spline weight of tap m."""
    return np.array(
        [
            [1 / 6, 4 / 6, 1 / 6, 0.0],
            [-3 / 6, 0.0, 3 / 6, 0.0],
            [3 / 6, -6 / 6, 3 / 6, 0.0],
            [-1 / 6, 3 / 6, -3 / 6, 1 / 6],
        ],
        dtype=np.float64,
    )


def build_W_const():
    """W[y, k*125+yc] = A[k, y-yc] for 0 <= y-yc <= 3 else 0. Shape [128, 500]."""
    A = bspline_poly_A()
    W = np.zeros((128, 4, 125), np.float32)
    for yc in range(NCELL):
        for m in range(4):
            for k in range(4):
                W[yc + m, k, yc] = A[k, m]
    return W.reshape(128, 500).astype(np.float16)


def _ap(t, offset, dims):
    """Build a raw AP on the same tensor as AP `t` with explicit [step, num] dims."""
    return bass.AP(tensor=t.tensor, offset=t.offset + offset, ap=[list(d) for d in dims])


@with_exitstack
def bspline_kernel(ctx: ExitStack, tc: tile.TileContext, outs, ins):
    """outs = [T_out [128, COLS, 3] f32]; ins = [xs, ys, zs [128, COLS] f32, phi [128,128,384] f32]."""
    nc = tc.nc
    xs, ys, zs, phi = ins
    t_out = outs[0]

    w_np = build_W_const()
    w_dram = nc.inline_tensor(w_np, name="w_const")

    dram = ctx.enter_context(tc.tile_pool(name="cydram", bufs=1, space="DRAM"))
    cy = dram.tile([NRECTOT, REC], F16)

    add = mybir.AluOpType.add
    sub = mybir.AluOpType.subtract
    mult = mybir.AluOpType.mult
    amin = mybir.AluOpType.min

    # ---------------- Phase 1: build Cy4 ----------------
    with ExitStack() as p1:
        singles = p1.enter_context(tc.tile_pool(name="p1_singles", bufs=1))
        phis = p1.enter_context(tc.tile_pool(name="p1_phi", bufs=3))
        stages = p1.enter_context(tc.tile_pool(name="p1_stage", bufs=6))
        psums = p1.enter_context(tc.psum_pool(name="p1_psum", bufs=2))

        w_sb = singles.tile([128, 500], F16)
        nc.sync.dma_start(out=w_sb[:], in_=w_dram.ap())

        def slot_ap(st, xsl):
            return _ap(st[:], xsl * 12, [[ROWE, NCELL], [REC, NX], [1, 12]])

        stage_by_x0 = {}
        for x2 in range(NX // 2):
            # cast fp32 -> fp16 during DMA (SWDGE); two x-slabs per load
            phi_x = phis.tile([128, 2 * ZC], F16, name="phi_in")
            nc.gpsimd.dma_start(
                out=phi_x[:],
                in_=_ap(phi, 2 * x2 * NX * ZC,
                        [[ZC, 128], [NX * ZC, 2], [1, ZC]]))
            for xh in range(2):
                x = 2 * x2 + xh
                ps = psums.tile([NCELL, 2048], F32)
                for k in range(4):
                    nc.tensor.matmul(
                        ps[:, k * 512:k * 512 + ZC],
                        w_sb[:, k * NCELL:(k + 1) * NCELL],
                        phi_x[:, xh * ZC:(xh + 1) * ZC],
                        start=True,
                        stop=True,
                    )
                targets = [(x - xsl, xsl) for xsl in range(4)
                           if 0 <= x - xsl <= NCELL - 1]
                for x0, xsl in targets:
                    if x0 not in stage_by_x0:
                        stage_by_x0[x0] = stages.tile([128, ROWE], F16, name="stage")
                # first target: direct fused fp32 psum -> fp16 stage copy (DVE)
                fx0, fxsl = targets[0]
                fst = stage_by_x0[fx0]
                nc.vector.tensor_copy(
                    out=_ap(fst[:], fxsl * 12,
                            [[ROWE, NCELL], [1, 4], [REC, NX], [4, NC_]]),
                    in_=_ap(ps[:], 0, [[2048, NCELL], [512, 4], [3, NX], [1, NC_]]))
                # remaining targets: cheap fp16 stage->stage copies (DVE/ACT)
                for i, (x0, xsl) in enumerate(targets[1:]):
                    st = stage_by_x0[x0]
                    if i == 0:
                        nc.vector.tensor_copy(
                            out=slot_ap(st, xsl), in_=slot_ap(fst, fxsl))
                    else:
                        nc.scalar.copy(
                            out=slot_ap(st, xsl), in_=slot_ap(fst, fxsl))
                # ship completed stages: stage x0 is complete once x == x0+3
                ship = []
                if x >= 3:
                    ship.append(x - 3)
                if x == NX - 1:
                    ship.extend([NCELL - 3, NCELL - 2, NCELL - 1])
                for x0 in ship:
                    if x0 not in stage_by_x0:
                        continue
                    st = stage_by_x0.pop(x0)
                    eng = nc.sync if (x0 % 2 == 0) else nc.gpsimd
                    eng.dma_start(
                        out=_ap(cy[:], x0 * XSTRIDE * REC,
                                [[ROWE, NCELL], [1, ROWE]]),
                        in_=_ap(st[:], 0, [[ROWE, NCELL], [1, ROWE]]),
                    )

    # ---------------- Phase 2: points ----------------
    with ExitStack() as p2:
        coords = p2.enter_context(tc.tile_pool(name="p2_coords", bufs=2))
        small = p2.enter_context(tc.tile_pool(name="p2_small", bufs=2))
        idxp = p2.enter_context(tc.tile_pool(name="p2_idx", bufs=2))
        recs = p2.enter_context(tc.tile_pool(name="p2_rec", bufs=2))
        prods = p2.enter_context(tc.tile_pool(name="p2_prod", bufs=2))
        touts = p2.enter_context(tc.tile_pool(name="p2_tout", bufs=2))

        cy_flat = _ap(cy[:], 0, [[REC, NRECTOT], [1, REC]])

        for ch in range(NCHUNK):
            # coords layout: [x | z | y] so (u,w) are adjacent for weights
            c3 = coords.tile([128, 3 * P], F32)
            nc.sync.dma_start(out=c3[:, 0:P], in_=xs[:, ch * P:(ch + 1) * P])
            nc.sync.dma_start(out=c3[:, P:2 * P], in_=zs[:, ch * P:(ch + 1) * P])
            nc.sync.dma_start(out=c3[:, 2 * P:3 * P], in_=ys[:, ch * P:(ch + 1) * P])

            # u = (coord+1)*62.5 ; fl = floor(u) (u >= 0) ; fr = u - fl
            nc.vector.tensor_scalar(c3[:], c3[:], 1.0, INV_D, add, mult)
            ci3 = small.tile([128, 3 * P], I32)
            nc.vector.tensor_copy(out=ci3[:], in_=c3[:])
            cf3 = small.tile([128, 3 * P], F32)
            nc.vector.tensor_copy(out=cf3[:], in_=ci3[:])
            fl3 = small.tile([128, 3 * P], F32)
            nc.vector.tensor_tensor(fl3[:], cf3[:], c3[:], mybir.AluOpType.is_gt)
            nc.vector.tensor_tensor(fl3[:], cf3[:], fl3[:], sub)
            fr3 = small.tile([128, 3 * P], F32)
            nc.vector.tensor_tensor(fr3[:], c3[:], fl3[:], sub)
            nc.vector.tensor_scalar(fl3[:], fl3[:], float(NCELL - 1), None, amin)
            frh = small.tile([128, 3 * P], F16)
            nc.vector.tensor_copy(out=frh[:], in_=fr3[:])

            # record index: ix*16000 + iy*128 + iz
            idxf = small.tile([128, P], F32)
            nc.vector.scalar_tensor_tensor(
                idxf[:], fl3[:, 2 * P:3 * P], float(NX), fl3[:, P:2 * P], mult, add)
            nc.vector.scalar_tensor_tensor(
                idxf[:], fl3[:, 0:P], float(XSTRIDE), idxf[:], mult, add)
            idxi = idxp.tile([128, P], I32)
            nc.vector.tensor_copy(out=idxi[:], in_=idxf[:])

            # ---- tap weights for u (x) and w (z): wt [128 | g2, P, k4] fp16 ----
            wt = small.tile([128, 2 * P * 4], F16)
            uw = _ap(frh[:], 0, [[3 * P, 128], [1, 2 * P]])

            def wslice(k):
                return _ap(wt[:], k, [[8 * P, 128], [4 * P, 2], [4, P]])

            tg = small.tile([128, 2 * P], F16)
            t2g = small.tile([128, 2 * P], F16)
            r2 = small.tile([128, 2 * P], F16)
            r3 = small.tile([128, 2 * P], F16)
            tmp = small.tile([128, 2 * P], F16)

            def v2(t):  # view [128, 2P] as (2, P)
                return _ap(t[:], 0, [[2 * P, 128], [P, 2], [1, P]])

            nc.vector.tensor_scalar(tg[:], uw, -1.0, 1.0, mult, add)
            nc.vector.tensor_tensor(t2g[:], tg[:], tg[:], mult)
            nc.vector.scalar_tensor_tensor(wslice(0), v2(t2g), 1 / 6, v2(tg), mult, mult)
            nc.vector.tensor_tensor(r2[:], uw, uw, mult)
            nc.vector.tensor_tensor(r3[:], r2[:], uw, mult)
            nc.vector.tensor_scalar(wslice(3), v2(r3), 1 / 6, None, mult)
            nc.vector.scalar_tensor_tensor(tmp[:], r3[:], 0.5, r2[:], mult, sub)
            nc.vector.tensor_scalar(wslice(1), v2(tmp), 2 / 3, None, add)
            nc.vector.tensor_tensor(v2(tmp), wslice(0), wslice(1), add)
            nc.vector.tensor_tensor(v2(tmp), v2(tmp), wslice(3), add)
            nc.vector.tensor_scalar(wslice(2), v2(tmp), -1.0, 1.0, mult, add)

            # ---- v powers: vp4 [128, P, 4] = [1, v, v^2, v^3] fp16 ----
            vp4 = small.tile([128, P * 4], F16)
            frv = _ap(frh[:], 2 * P, [[3 * P, 128], [1, P]])

            def vslot(k):
                return _ap(vp4[:], k, [[4 * P, 128], [4, P]])

            nc.vector.memset(vslot(0), 1.0)
            nc.vector.tensor_copy(out=vslot(1), in_=frv)
            nc.vector.tensor_tensor(vslot(2), frv, frv, mult)
            nc.vector.tensor_tensor(vslot(3), vslot(2), frv, mult)

            # ---- x weights expanded over c: wuc [128, P, x4, c3] fp16 ----
            wuc = small.tile([128, P * 12], F16)
            nc.vector.tensor_copy(
                out=_ap(wuc[:], 0, [[12 * P, 128], [12, P], [3, 4], [1, 3]]),
                in_=_ap(wt[:], 0, [[8 * P, 128], [4, P], [1, 4], [0, 3]]))

            # ---- gather: one record (z-window, 192 fp16) per point ----
            rec = recs.tile([128, P * 192], F16)
            for t in range(P):
                nc.gpsimd.indirect_dma_start(
                    out=_ap(rec[:], t * 192, [[192 * P, 128], [1, 192]]),
                    out_offset=None,
                    in_=cy_flat,
                    in_offset=bass.IndirectOffsetOnAxis(
                        ap=_ap(idxi[:], t, [[P, 128], [1, 1]]), axis=0),
                )

            # ---- contraction ----
            # per point rec = [z4][x4][c3][k4]
            # k poly-eval: rec[pt, zxc48, k4] *= vp4[pt, k4]; tree-add over k
            nc.vector.tensor_tensor(
                _ap(rec[:], 0, [[192 * P, 128], [192, P], [4, 48], [1, 4]]),
                _ap(rec[:], 0, [[192 * P, 128], [192, P], [4, 48], [1, 4]]),
                _ap(vp4[:], 0, [[4 * P, 128], [4, P], [0, 48], [1, 4]]),
                mult)
            nc.vector.tensor_tensor(
                _ap(rec[:], 0, [[192 * P, 128], [192, P], [4, 48], [1, 2]]),
                _ap(rec[:], 0, [[192 * P, 128], [192, P], [4, 48], [1, 2]]),
                _ap(rec[:], 2, [[192 * P, 128], [192, P], [4, 48], [1, 2]]),
                add)
            s1 = prods.tile([128, P * 48], F16)
            nc.vector.tensor_tensor(
                _ap(s1[:], 0, [[48 * P, 128], [48, P], [1, 48]]),
                _ap(rec[:], 0, [[192 * P, 128], [192, P], [4, 48]]),
                _ap(rec[:], 1, [[192 * P, 128], [192, P], [4, 48]]),
                add)
            # x contraction: s1[pt, z4, (x4 c3)12] *= wuc; tree-add over x
            nc.vector.tensor_tensor(
                _ap(s1[:], 0, [[48 * P, 128], [48, P], [12, 4], [1, 12]]),
                _ap(s1[:], 0, [[48 * P, 128], [48, P], [12, 4], [1, 12]]),
                _ap(wuc[:], 0, [[12 * P, 128], [12, P], [0, 4], [1, 12]]),
                mult)
            nc.vector.tensor_tensor(
                _ap(s1[:], 0, [[48 * P, 128], [48, P], [12, 4], [1, 6]]),
                _ap(s1[:], 0, [[48 * P, 128], [48, P], [12, 4], [1, 6]]),
                _ap(s1[:], 6, [[48 * P, 128], [48, P], [12, 4], [1, 6]]),
                add)
            s2 = prods.tile([128, P * 12], F16)
            nc.vector.tensor_tensor(
                _ap(s2[:], 0, [[12 * P, 128], [12, P], [3, 4], [1, 3]]),
                _ap(s1[:], 0, [[48 * P, 128], [48, P], [12, 4], [1, 3]]),
                _ap(s1[:], 3, [[48 * P, 128], [48, P], [12, 4], [1, 3]]),
                add)
            # z contraction: s2[pt, z4, c3] *= ww (bcast over c); tree-add over z
            nc.vector.tensor_tensor(
                _ap(s2[:], 0, [[12 * P, 128], [12, P], [3, 4], [1, 3]]),
                _ap(s2[:], 0, [[12 * P, 128], [12, P], [3, 4], [1, 3]]),
                _ap(wt[:], 4 * P, [[8 * P, 128], [4, P], [1, 4], [0, 3]]),
                mult)
            nc.vector.tensor_tensor(
                _ap(s2[:], 0, [[12 * P, 128], [12, P], [1, 6]]),
                _ap(s2[:], 0, [[12 * P, 128], [12, P], [1, 6]]),
                _ap(s2[:], 6, [[12 * P, 128], [12, P], [1, 6]]),
                add)
            t_c = touts.tile([128, P * 3], F32)
            nc.vector.tensor_tensor(
                _ap(t_c[:], 0, [[3 * P, 128], [3, P], [1, 3]]),
                _ap(s2[:], 0, [[12 * P, 128], [12, P], [1, 3]]),
                _ap(s2[:], 3, [[12 * P, 128], [12, P], [1, 3]]),
                add)

            nc.sync.dma_start(
                out=t_out[:, ch * P:(ch + 1) * P, :],
                in_=t_c[:].rearrange("p (a b) -> p a b", b=3))


# ======================================================================
# Self-contained entry point: kernel(**inputs) -> np.ndarray
# ======================================================================

N_POINTS = 2_000_000
N_CORES = 8
PTS_PER_CORE = N_POINTS // N_CORES      # 250000
PAD_PER_CORE = 128 * COLS               # 262144

_CACHE = {}


def _build_nc():
    import concourse.bacc as bacc

    nc = bacc.Bacc(
        "TRN2",
        target_bir_lowering=False,
        debug=False,
        num_devices=N_CORES,
    )
    xs = nc.dram_tensor("xs", [128, COLS], F32, kind="ExternalInput").ap()
    ys = nc.dram_tensor("ys", [128, COLS], F32, kind="ExternalInput").ap()
    zs = nc.dram_tensor("zs", [128, COLS], F32, kind="ExternalInput").ap()
    phi = nc.dram_tensor("phi", [128, 128, ZC], F32, kind="ExternalInput").ap()
    t_out = nc.dram_tensor("t_out", [128, COLS, NC_], F32, kind="ExternalOutput").ap()

    with tile.TileContext(nc) as tc:
        bspline_kernel(tc, [t_out], [xs, ys, zs, phi])
    nc.compile()
    return nc


def get_nc():
    if "nc" not in _CACHE:
        _CACHE["nc"] = _build_nc()
    return _CACHE["nc"]


def _shard(arr):
    """[N_POINTS] -> list of 8 [128, COLS] arrays (padded with zeros)."""
    out = []
    for c in range(N_CORES):
        s = arr[c * PTS_PER_CORE:(c + 1) * PTS_PER_CORE]
        p = np.zeros(PAD_PER_CORE, dtype=np.float32)
        p[:PTS_PER_CORE] = s
        out.append(p.reshape(128, COLS))
    return out


def run_on_cores(x, y, z, phi_x, trace=False, **kw):
    from concourse.bass_utils import run_bass_kernel_spmd

    nc = get_nc()
    xsh, ysh, zsh = _shard(x), _shard(y), _shard(z)
    phi_r = np.ascontiguousarray(phi_x.reshape(128, 128, ZC))
    in_maps = [
        {"xs": xsh[c], "ys": ysh[c], "zs": zsh[c], "phi": phi_r}
        for c in range(N_CORES)
    ]
    res = run_bass_kernel_spmd(
        nc, in_maps, core_ids=list(range(N_CORES)), trace=trace, **kw
    )
    outs = []
    for c in range(N_CORES):
        t = res.results[c]["t_out"].reshape(PAD_PER_CORE, NC_)
        outs.append(t[:PTS_PER_CORE])
    full = np.concatenate(outs, axis=0).astype(np.float32)
    return full, res


def kernel(x, y, z, phi_x):
    full, _ = run_on_cores(
        np.asarray(x, dtype=np.float32),
        np.asarray(y, dtype=np.float32),
        np.asarray(z, dtype=np.float32),
        np.asarray(phi_x, dtype=np.float32),
    )
    return full


# revision 12
# speedup vs baseline: 2.2096x; 1.0780x over previous
"""Bass/Tile kernel builder for BSplineField3d (tricubic B-spline interpolation).

Algorithm (per NeuronCore, data-parallel over points):
  Phase 1 (build): from phi [128,128,128,3] build Cy4 in fp16:
      Cy4[x0, yc, z, xs, c, k] = sum_m A[k,m] * phi[x0+xs, yc+m, z, c]
    (x0 in [0,124], xs in [0,4)).  The y-dim B-spline is pre-contracted into
    per-cell polynomial coefficients in v; the 4 x-taps of a point are
    DUPLICATED into every record so that one point needs exactly ONE
    contiguous gather: records of 48 fp16 = [xs4][c3][k4] are contiguous
    along z, so the z-window (4 records = 192 fp16 = 384 B) starting at
    (x0=ix, yc=iy, z=iz) holds everything point-specific.
    Built with fp16 PE matmuls against a banded matrix W[y,(k,yc)], with a
    sliding window of stage tiles (each x-slab feeds 4 stages).
  Phase 2 (points): per chunk of 128x128 points:
    - cell indices + fractional coords on DVE
    - P indirect-DMA gathers (one index per partition per instruction,
      the only vector-mode the HW ucode supports), 384 B per descriptor
    - contraction on DVE in fp16 (packed APs -> 2x perf mode):
        poly-eval in v over k (mult + tree-add), weighted x taps,
        weighted z taps (tree-adds, partially in-place)
"""

from contextlib import ExitStack

import sys as _sys
for _p in ("/opt/trn_rl_repo",):
    if _p not in _sys.path:
        _sys.path.append(_p)

import numpy as np

import concourse.bass as bass
import concourse.tile as tile
from concourse import mybir
from concourse._compat import with_exitstack

F32 = mybir.dt.float32
F16 = mybir.dt.float16
I32 = mybir.dt.int32

NX = 128          # grid points per dim
NCELL = 125       # valid cells per dim (ix in [0,124])
NC_ = 3           # components
ZC = NX * NC_     # 384 floats per (x,y) z-row in phi
REC = 48          # [xs4][c3][k4] fp16 per (x0,yc,z) record in Cy4
ROWE = NX * REC   # 6144 fp16 per (x0,yc)
NRECTOT = NCELL * NCELL * NX   # 2,000,000 records
XSTRIDE = NCELL * NX           # 16000: record-index stride for x0

COLS = 1984       # points per partition (128*1984 = 253952 >= 250000)
P = 124           # points per partition per chunk
NCHUNK = COLS // P  # 16

# spacing: dx = 2/(nx-3) = 2/125 -> 1/dx = 62.5; u = (x+1)*62.5
INV_D = 62.5


def bspline_poly_A():
    """A[k][m]: coefficient of v^k in the cubic B-spline weight of tap m."""
    return np.array(
        [
            [1 / 6, 4 / 6, 1 / 6, 0.0],
            [-3 / 6, 0.0, 3 / 6, 0.0],
            [3 / 6, -6 / 6, 3 / 6, 0.0],
            [-1 / 6, 3 / 6, -3 / 6, 1 / 6],
        ],
        dtype=np.float64,
    )


def build_W_const():
    """W[y, k*125+yc] = A[k, y-yc] for 0 <= y-yc <= 3 else 0. Shape [128, 500]."""
    A = bspline_poly_A()
    W = np.zeros((128, 4, 125), np.float32)
    for yc in range(NCELL):
        for m in range(4):
            for k in range(4):
                W[yc + m, k, yc] = A[k, m]
    return W.reshape(128, 500).astype(np.float16)


def _ap(t, offset, dims):
    """Build a raw AP on the same tensor as AP `t` with explicit [step, num] dims."""
    return bass.AP(tensor=t.tensor, offset=t.offset + offset, ap=[list(d) for d in dims])


@with_exitstack
def bspline_kernel(ctx: ExitStack, tc: tile.TileContext, outs, ins):
    """outs = [T_out [128, COLS, 3] f32]; ins = [xs, ys, zs [128, COLS] f32, phi [128,128,384] f32]."""
    nc = tc.nc
    xs, ys, zs, phi = ins
    t_out = outs[0]

    w_np = build_W_const()
    w_dram = nc.inline_tensor(w_np, name="w_const")

    dram = ctx.enter_context(tc.tile_pool(name="cydram", bufs=1, space="DRAM"))
    cy = dram.tile([NRECTOT, REC], F16)

    add = mybir.AluOpType.add
    sub = mybir.AluOpType.subtract
    mult = mybir.AluOpType.mult
    amin = mybir.AluOpType.min

    # phase-2 prep pools opened early so chunk prep can overlap phase 1
    coords = ctx.enter_context(tc.tile_pool(name="p2_coords", bufs=2))
    small = ctx.enter_context(tc.tile_pool(name="p2_small", bufs=2))
    idxp = ctx.enter_context(tc.tile_pool(name="p2_idx", bufs=2))

    # ---------------- Phase 1: build Cy4 ----------------
    with ExitStack() as p1:
        singles = p1.enter_context(tc.tile_pool(name="p1_singles", bufs=1))
        phis = p1.enter_context(tc.tile_pool(name="p1_phi", bufs=4))
        stages = p1.enter_context(tc.tile_pool(name="p1_stage", bufs=8))
        psums = p1.enter_context(tc.psum_pool(name="p1_psum", bufs=2))

        w_sb = singles.tile([128, 500], F16)
        nc.sync.dma_start(out=w_sb[:], in_=w_dram.ap())

        def slot_ap(st, xsl):
            return _ap(st[:], xsl * 12, [[ROWE, NCELL], [REC, NX], [1, 12]])

        stage_by_x0 = {}
        for x2 in range(NX // 2):
            # cast fp32 -> fp16 during DMA (SWDGE); two x-slabs per load
            phi_x = phis.tile([128, 2 * ZC], F16, name="phi_in")
            nc.gpsimd.dma_start(
                out=phi_x[:],
                in_=_ap(phi, 2 * x2 * NX * ZC,
                        [[ZC, 128], [NX * ZC, 2], [1, ZC]]))
            for xh in range(2):
                x = 2 * x2 + xh
                ps = psums.tile([NCELL, 2048], F32)
                for k in range(4):
                    nc.tensor.matmul(
                        ps[:, k * 512:k * 512 + ZC],
                        w_sb[:, k * NCELL:(k + 1) * NCELL],
                        phi_x[:, xh * ZC:(xh + 1) * ZC],
                        start=True,
                        stop=True,
                    )
                targets = [(x - xsl, xsl) for xsl in range(4)
                           if 0 <= x - xsl <= NCELL - 1]
                for x0, xsl in targets:
                    if x0 not in stage_by_x0:
                        stage_by_x0[x0] = stages.tile([128, ROWE], F16, name="stage")
                # first target: direct fused fp32 psum -> fp16 stage copy (DVE)
                fx0, fxsl = targets[0]
                fst = stage_by_x0[fx0]
                nc.vector.tensor_copy(
                    out=_ap(fst[:], fxsl * 12,
                            [[ROWE, NCELL], [1, 4], [REC, NX], [4, NC_]]),
                    in_=_ap(ps[:], 0, [[2048, NCELL], [512, 4], [3, NX], [1, NC_]]))
                # remaining targets: cheap fp16 stage->stage copies (DVE/ACT)
                for i, (x0, xsl) in enumerate(targets[1:]):
                    st = stage_by_x0[x0]
                    if i == 0:
                        nc.vector.tensor_copy(
                            out=slot_ap(st, xsl), in_=slot_ap(fst, fxsl))
                    else:
                        nc.scalar.copy(
                            out=slot_ap(st, xsl), in_=slot_ap(fst, fxsl))
                # ship completed stages: stage x0 is complete once x == x0+3
                ship = []
                if x >= 3:
                    ship.append(x - 3)
                if x == NX - 1:
                    ship.extend([NCELL - 3, NCELL - 2, NCELL - 1])
                for x0 in ship:
                    if x0 not in stage_by_x0:
                        continue
                    st = stage_by_x0.pop(x0)
                    eng = nc.sync if (x0 % 2 == 0) else nc.gpsimd
                    eng.dma_start(
                        out=_ap(cy[:], x0 * XSTRIDE * REC,
                                [[ROWE, NCELL], [1, ROWE]]),
                        in_=_ap(st[:], 0, [[ROWE, NCELL], [1, ROWE]]),
                    )

    # ---------------- Phase 2: points ----------------
    with ExitStack() as p2:
        recs = p2.enter_context(tc.tile_pool(name="p2_rec", bufs=2))
        prods = p2.enter_context(tc.tile_pool(name="p2_prod", bufs=2))
        touts = p2.enter_context(tc.tile_pool(name="p2_tout", bufs=2))

        cy_flat = _ap(cy[:], 0, [[REC, NRECTOT], [1, REC]])

        for ch in range(NCHUNK):
            # coords layout: [x | z | y] so (u,w) are adjacent for weights
            c3 = coords.tile([128, 3 * P], F32)
            nc.sync.dma_start(out=c3[:, 0:P], in_=xs[:, ch * P:(ch + 1) * P])
            nc.sync.dma_start(out=c3[:, P:2 * P], in_=zs[:, ch * P:(ch + 1) * P])
            nc.sync.dma_start(out=c3[:, 2 * P:3 * P], in_=ys[:, ch * P:(ch + 1) * P])

            # u = (coord+1)*62.5 ; fl = floor(u) (u >= 0) ; fr = u - fl
            nc.vector.tensor_scalar(c3[:], c3[:], 1.0, INV_D, add, mult)
            ci3 = small.tile([128, 3 * P], I32)
            nc.vector.tensor_copy(out=ci3[:], in_=c3[:])
            cf3 = small.tile([128, 3 * P], F32)
            nc.vector.tensor_copy(out=cf3[:], in_=ci3[:])
            fl3 = small.tile([128, 3 * P], F32)
            nc.vector.tensor_tensor(fl3[:], cf3[:], c3[:], mybir.AluOpType.is_gt)
            nc.vector.tensor_tensor(fl3[:], cf3[:], fl3[:], sub)
            fr3 = small.tile([128, 3 * P], F32)
            nc.vector.tensor_tensor(fr3[:], c3[:], fl3[:], sub)
            nc.vector.tensor_scalar(fl3[:], fl3[:], float(NCELL - 1), None, amin)
            frh = small.tile([128, 3 * P], F16)
            nc.vector.tensor_copy(out=frh[:], in_=fr3[:])

            # record index: ix*16000 + iy*128 + iz
            idxf = small.tile([128, P], F32)
            nc.vector.scalar_tensor_tensor(
                idxf[:], fl3[:, 2 * P:3 * P], float(NX), fl3[:, P:2 * P], mult, add)
            nc.vector.scalar_tensor_tensor(
                idxf[:], fl3[:, 0:P], float(XSTRIDE), idxf[:], mult, add)
            idxi = idxp.tile([128, P], I32)
            nc.vector.tensor_copy(out=idxi[:], in_=idxf[:])

            # ---- tap weights for u (x) and w (z): wt [128 | g2, P, k4] fp16 ----
            wt = small.tile([128, 2 * P * 4], F16)
            uw = _ap(frh[:], 0, [[3 * P, 128], [1, 2 * P]])

            def wslice(k):
                return _ap(wt[:], k, [[8 * P, 128], [4 * P, 2], [4, P]])

            tg = small.tile([128, 2 * P], F16)
            t2g = small.tile([128, 2 * P], F16)
            r2 = small.tile([128, 2 * P], F16)
            r3 = small.tile([128, 2 * P], F16)
            tmp = small.tile([128, 2 * P], F16)

            def v2(t):  # view [128, 2P] as (2, P)
                return _ap(t[:], 0, [[2 * P, 128], [P, 2], [1, P]])

            nc.vector.tensor_scalar(tg[:], uw, -1.0, 1.0, mult, add)
            nc.vector.tensor_tensor(t2g[:], tg[:], tg[:], mult)
            nc.vector.scalar_tensor_tensor(wslice(0), v2(t2g), 1 / 6, v2(tg), mult, mult)
            nc.vector.tensor_tensor(r2[:], uw, uw, mult)
            nc.vector.tensor_tensor(r3[:], r2[:], uw, mult)
            nc.vector.tensor_scalar(wslice(3), v2(r3), 1 / 6, None, mult)
            nc.vector.scalar_tensor_tensor(tmp[:], r3[:], 0.5, r2[:], mult, sub)
            nc.vector.tensor_scalar(wslice(1), v2(tmp), 2 / 3, None, add)
            nc.vector.tensor_tensor(v2(tmp), wslice(0), wslice(1), add)
            nc.vector.tensor_tensor(v2(tmp), v2(tmp), wslice(3), add)
            nc.vector.tensor_scalar(wslice(2), v2(tmp), -1.0, 1.0, mult, add)

            # ---- v powers: vp4 [128, P, 4] = [1, v, v^2, v^3] fp16 ----
            vp4 = small.tile([128, P * 4], F16)
            frv = _ap(frh[:], 2 * P, [[3 * P, 128], [1, P]])

            def vslot(k):
                return _ap(vp4[:], k, [[4 * P, 128], [4, P]])

            nc.vector.memset(vslot(0), 1.0)
            nc.vector.tensor_copy(out=vslot(1), in_=frv)
            nc.vector.tensor_tensor(vslot(2), frv, frv, mult)
            nc.vector.tensor_tensor(vslot(3), vslot(2), frv, mult)

            # ---- x weights expanded over c: wuc [128, P, x4, c3] fp16 ----
            wuc = small.tile([128, P * 12], F16)
            nc.vector.tensor_copy(
                out=_ap(wuc[:], 0, [[12 * P, 128], [12, P], [3, 4], [1, 3]]),
                in_=_ap(wt[:], 0, [[8 * P, 128], [4, P], [1, 4], [0, 3]]))

            # ---- gather: one record (z-window, 192 fp16) per point ----
            rec = recs.tile([128, P * 192], F16)
            for t in range(P):
                nc.gpsimd.indirect_dma_start(
                    out=_ap(rec[:], t * 192, [[192 * P, 128], [1, 192]]),
                    out_offset=None,
                    in_=cy_flat,
                    in_offset=bass.IndirectOffsetOnAxis(
                        ap=_ap(idxi[:], t, [[P, 128], [1, 1]]), axis=0),
                )

            # ---- contraction ----
            # per point rec = [z4][x4][c3][k4]
            # k poly-eval: rec[pt, zxc48, k4] *= vp4[pt, k4]; tree-add over k
            nc.vector.tensor_tensor(
                _ap(rec[:], 0, [[192 * P, 128], [192, P], [4, 48], [1, 4]]),
                _ap(rec[:], 0, [[192 * P, 128], [192, P], [4, 48], [1, 4]]),
                _ap(vp4[:], 0, [[4 * P, 128], [4, P], [0, 48], [1, 4]]),
                mult)
            nc.vector.tensor_tensor(
                _ap(rec[:], 0, [[192 * P, 128], [192, P], [4, 48], [1, 2]]),
                _ap(rec[:], 0, [[192 * P, 128], [192, P], [4, 48], [1, 2]]),
                _ap(rec[:], 2, [[192 * P, 128], [192, P], [4, 48], [1, 2]]),
                add)
            s1 = prods.tile([128, P * 48], F16)
            nc.vector.tensor_tensor(
                _ap(s1[:], 0, [[48 * P, 128], [48, P], [1, 48]]),
                _ap(rec[:], 0, [[192 * P, 128], [192, P], [4, 48]]),
                _ap(rec[:], 1, [[192 * P, 128], [192, P], [4, 48]]),
                add)
            # x contraction: s1[pt, z4, (x4 c3)12] *= wuc; tree-add over x
            nc.vector.tensor_tensor(
                _ap(s1[:], 0, [[48 * P, 128], [48, P], [12, 4], [1, 12]]),
                _ap(s1[:], 0, [[48 * P, 128], [48, P], [12, 4], [1, 12]]),
                _ap(wuc[:], 0, [[12 * P, 128], [12, P], [0, 4], [1, 12]]),
                mult)
            nc.vector.tensor_tensor(
                _ap(s1[:], 0, [[48 * P, 128], [48, P], [12, 4], [1, 6]]),
                _ap(s1[:], 0, [[48 * P, 128], [48, P], [12, 4], [1, 6]]),
                _ap(s1[:], 6, [[48 * P, 128], [48, P], [12, 4], [1, 6]]),
                add)
            s2 = prods.tile([128, P * 12], F16)
            nc.vector.tensor_tensor(
                _ap(s2[:], 0, [[12 * P, 128], [12, P], [3, 4], [1, 3]]),
                _ap(s1[:], 0, [[48 * P, 128], [48, P], [12, 4], [1, 3]]),
                _ap(s1[:], 3, [[48 * P, 128], [48, P], [12, 4], [1, 3]]),
                add)
            # z contraction: s2[pt, z4, c3] *= ww (bcast over c); tree-add over z
            nc.vector.tensor_tensor(
                _ap(s2[:], 0, [[12 * P, 128], [12, P], [3, 4], [1, 3]]),
                _ap(s2[:], 0, [[12 * P, 128], [12, P], [3, 4], [1, 3]]),
                _ap(wt[:], 4 * P, [[8 * P, 128], [4, P], [1, 4], [0, 3]]),
                mult)
            nc.vector.tensor_tensor(
                _ap(s2[:], 0, [[12 * P, 128], [12, P], [1, 6]]),
                _ap(s2[:], 0, [[12 * P, 128], [12, P], [1, 6]]),
                _ap(s2[:], 6, [[12 * P, 128], [12, P], [1, 6]]),
                add)
            t_c = touts.tile([128, P * 3], F32)
            nc.vector.tensor_tensor(
                _ap(t_c[:], 0, [[3 * P, 128], [3, P], [1, 3]]),
                _ap(s2[:], 0, [[12 * P, 128], [12, P], [1, 3]]),
                _ap(s2[:], 3, [[12 * P, 128], [12, P], [1, 3]]),
                add)

            nc.sync.dma_start(
                out=t_out[:, ch * P:(ch + 1) * P, :],
                in_=t_c[:].rearrange("p (a b) -> p a b", b=3))


# ======================================================================
# Self-contained entry point: kernel(**inputs) -> np.ndarray
# ======================================================================

N_POINTS = 2_000_000
N_CORES = 8
PTS_PER_CORE = N_POINTS // N_CORES      # 250000
PAD_PER_CORE = 128 * COLS               # 262144

_CACHE = {}


def _build_nc():
    import concourse.bacc as bacc

    nc = bacc.Bacc(
        "TRN2",
        target_bir_lowering=False,
        debug=False,
        num_devices=N_CORES,
    )
    xs = nc.dram_tensor("xs", [128, COLS], F32, kind="ExternalInput").ap()
    ys = nc.dram_tensor("ys", [128, COLS], F32, kind="ExternalInput").ap()
    zs = nc.dram_tensor("zs", [128, COLS], F32, kind="ExternalInput").ap()
    phi = nc.dram_tensor("phi", [128, 128, ZC], F32, kind="ExternalInput").ap()
    t_out = nc.dram_tensor("t_out", [128, COLS, NC_], F32, kind="ExternalOutput").ap()

    with tile.TileContext(nc) as tc:
        bspline_kernel(tc, [t_out], [xs, ys, zs, phi])
    nc.compile()
    return nc


def get_nc():
    if "nc" not in _CACHE:
        _CACHE["nc"] = _build_nc()
    return _CACHE["nc"]


def _shard(arr):
    """[N_POINTS] -> list of 8 [128, COLS] arrays (padded with zeros)."""
    out = []
    for c in range(N_CORES):
        s = arr[c * PTS_PER_CORE:(c + 1) * PTS_PER_CORE]
        p = np.zeros(PAD_PER_CORE, dtype=np.float32)
        p[:PTS_PER_CORE] = s
        out.append(p.reshape(128, COLS))
    return out


def run_on_cores(x, y, z, phi_x, trace=False, **kw):
    from concourse.bass_utils import run_bass_kernel_spmd

    nc = get_nc()
    xsh, ysh, zsh = _shard(x), _shard(y), _shard(z)
    phi_r = np.ascontiguousarray(phi_x.reshape(128, 128, ZC))
    in_maps = [
        {"xs": xsh[c], "ys": ysh[c], "zs": zsh[c], "phi": phi_r}
        for c in range(N_CORES)
    ]
    res = run_bass_kernel_spmd(
        nc, in_maps, core_ids=list(range(N_CORES)), trace=trace, **kw
    )
    outs = []
    for c in range(N_CORES):
        t = res.results[c]["t_out"].reshape(PAD_PER_CORE, NC_)
        outs.append(t[:PTS_PER_CORE])
    full = np.concatenate(outs, axis=0).astype(np.float32)
    return full, res


def kernel(x, y, z, phi_x):
    full, _ = run_on_cores(
        np.asarray(x, dtype=np.float32),
        np.asarray(y, dtype=np.float32),
        np.asarray(z, dtype=np.float32),
        np.asarray(phi_x, dtype=np.float32),
    )
    return full
